# revision 27
# baseline (speedup 1.0000x reference)
"""Trainium2 Bass kernel for nn_Attention_43868795961547 (sparse_attention).

Reference computation per batch item (8 items, data-parallel over 8 cores):
  x  = LN(img[b]) @ w_qkv -> q,k,v (8 heads x 64)          [1024 tokens]
  kt,vt from LN(tab[b]) @ w_tab_qkv appended as key/value position 1024
  out = softmax(q k^T / 8) @ v ; out @ w_out + b_out        -> [1024, 512]

v6 strategy (per core); ACT-exp is the roofline (~55us of exp at 1.2GHz):
  - Host passes img transposed (imgT) and folds ln_w/ln_b into the qkv /
    tab weights (W' = diag(ln_w) @ W, bias = ln_b @ W), so the kernel
    needs NO PE transposes of activations: LN stats are computed
    token-major, mean*rstd / rstd rows are broadcast down partitions by
    K=1 f32r ones-matmuls, and xnT = imgT*rstd_bc - musc_bc on DVE.
  - dots start as soon as the first two qkv column blocks finish; the
    whole rest of the kernel (remaining qkv, v, tab, attn@v, final)
    fills PE/DVE around the Scalar exp stream.
  - dots: two heads packed at PE row groups 0-63/64-127.
  - attn@v: M=65 (64 v cols + ones col -> softmax denominator for free),
    66-element head stride so bf16 slices stay 4B-aligned.
  - tab path via row-major matvecs + block-diagonal tab-dots.
  - ae / v / out^T / w_out in bf16.
  - softmax denominators: per head-pair reciprocal_approx_fast, broadcast
    down partitions via K=1 f32r ones-matmuls into psum, multiplied on
    DVE; no DRAM bounce, ~1.3us chain.
  - PSUM: 3x [128,1024] tags for dots/qkv + 2 rotating 1-bank tags for
    attn@v / final / tab / broadcasts.
"""

import numpy as np

import concourse.bass as bass
import concourse.mybir as mybir
import concourse.tile as tile
from concourse import bacc
from concourse import bass_utils
from concourse.masks import make_identity

F32 = mybir.dt.float32
F32R = mybir.dt.float32r
BF16 = mybir.dt.bfloat16

N_CORES = 8
NTOK = 1024  # img tokens per batch item
DIM = 512
HEADS = 8
DHEAD = 64
INNER = 512
SCALE = DHEAD ** -0.5  # 0.125
EPS = 1e-5

NT = NTOK // 128   # 8 token tiles
NC_ = DIM // 128   # 4 feature chunks
NQB = 2            # q blocks of 512
QB = 512

import os
KDBG = os.environ.get("KDBG", "") == "1"


def build_program():
    nc = bacc.Bacc(
        "TRN2",
        target_bir_lowering=False,
        debug=False,
        enable_asserts=False,
        num_devices=N_CORES,
    )

    img = nc.dram_tensor("img_s", [NTOK, DIM], F32, kind="ExternalInput").ap()
    imgT = nc.dram_tensor("imgT_s", [DIM, NTOK], F32, kind="ExternalInput").ap()
    tab = nc.dram_tensor("tab_s", [1, DIM], F32, kind="ExternalInput").ap()
    # ln_w-folded weights; w_tab = k,v columns only
    w_qkv = nc.dram_tensor("w_qkv", [DIM, 3 * INNER], F32, kind="ExternalInput").ap()
    w_tab = nc.dram_tensor("w_tab", [DIM, 2 * INNER], F32, kind="ExternalInput").ap()
    w_out = nc.dram_tensor("w_out", [INNER, DIM], F32, kind="ExternalInput").ap()
    b_out = nc.dram_tensor("b_out", [1, DIM], F32, kind="ExternalInput").ap()
    # ln_b @ w biases: feature-major columns for q/k, token-broadcast row
    # for v, row for tab k/v
    cqkvT = nc.dram_tensor("cqkvT_s", [128, 12], F32, kind="ExternalInput").ap()
    cv = nc.dram_tensor("cv_s", [1, INNER], F32, kind="ExternalInput").ap()
    ctab = nc.dram_tensor("ctab_s", [1, 2 * INNER], F32, kind="ExternalInput").ap()
    out_d = nc.dram_tensor("out_s", [NTOK, DIM], F32, kind="ExternalOutput").ap()
    dbg = None
    if KDBG:
        dbg = {n: nc.dram_tensor(n, [NTOK, DIM], F32, kind="ExternalOutput").ap()
               for n in ("dbg_qk", "dbg_ae", "dbg_att")}

    with tile.TileContext(nc) as tc:
        kernel_body(tc, img, imgT, tab, w_qkv, w_tab, w_out, b_out,
                    cqkvT, cv, ctab, out_d, dbg)

    nc.compile()
    return nc


def kernel_body(tc, img, imgT, tab, w_qkv, w_tab, w_out, b_out,
                cqkvT, cv, ctab, out_d, dbg=None):
    nc = tc.nc
    AF = mybir.ActivationFunctionType
    OP = mybir.AluOpType

    import contextlib
    ctx = contextlib.ExitStack()
    scoped = contextlib.ExitStack()  # pools freed once weights/imgT consumed
    with ctx:
        # ---------------- persistent pools ----------------
        const_p = ctx.enter_context(tc.tile_pool(name="const", bufs=1))
        qkT_p = ctx.enter_context(tc.tile_pool(name="qkT", bufs=1))
        v_p = ctx.enter_context(tc.tile_pool(name="vp", bufs=1))
        outuT_p = ctx.enter_context(tc.tile_pool(name="outuT", bufs=1))
        small_p = ctx.enter_context(tc.tile_pool(name="smallp", bufs=1))
        wo_p = ctx.enter_context(tc.tile_pool(name="wop", bufs=1))
        ae_p = ctx.enter_context(tc.tile_pool(name="aep", bufs=1))
        fo_p = ctx.enter_context(tc.tile_pool(name="fout", bufs=2))
        dbg_p = ctx.enter_context(tc.tile_pool(name="dbgp", bufs=1)) \
            if dbg is not None else None

        psum_big = ctx.enter_context(tc.tile_pool(name="psbig", bufs=1, space="PSUM"))
        psum_o = ctx.enter_context(tc.tile_pool(name="pso", bufs=1, space="PSUM"))
        bigctr = [0]
        octr = [0]

        def big_tile(name):
            t = psum_big.tile([128, 2 * QB], F32, name=name,
                              tag=f"big{bigctr[0] % 3}")
            bigctr[0] += 1
            return t

        def o_tile(shape, name):
            t = psum_o.tile(shape, F32, name=name, tag=f"o{octr[0] % 2}")
            octr[0] += 1
            return t

        # ---------------- constants + input DMAs ----------------
        ident = const_p.tile([128, 128], F32, name="ident")
        make_identity(nc, ident)

        eps_t = const_p.tile([128, 1], F32, name="eps_t")
        nc.vector.memset(eps_t, EPS)

        ones_f = const_p.tile([128, 64], F32, name="ones_f")
        nc.vector.memset(ones_f, 1.0)
        ones8 = const_p.tile([128, 8], BF16, name="ones8")
        nc.vector.tensor_copy(out=ones8, in_=ones_f[:, 0:8])
        ones_row = const_p.tile([128, 64], BF16, name="ones_row")
        nc.vector.tensor_copy(out=ones_row, in_=ones_f)
        ones_fr = const_p.tile([128, 128], F32R, name="ones_fr")
        nc.vector.memset(ones_fr.bitcast(F32), 1.0)

        bout_bc = const_p.tile([128, DIM], F32, name="bout_bc")
        cv_bc = const_p.tile([128, INNER], F32, name="cv_bc")
        cqkc = const_p.tile([128, 12], F32, name="cqkc")
        ctab_sb = const_p.tile([1, 2 * INNER], F32, name="ctab_sb")
        nc.gpsimd.dma_start(out=cqkc, in_=cqkvT)
        nc.gpsimd.dma_start(out=cv_bc, in_=cv.to_broadcast([128, INNER]))
        nc.gpsimd.dma_start(out=ctab_sb, in_=ctab)

        wq_p = scoped.enter_context(tc.tile_pool(name="wqkv", bufs=1))
        xi_p = scoped.enter_context(tc.tile_pool(name="xip", bufs=1))
        ln_p = scoped.enter_context(tc.tile_pool(name="lnt", bufs=2))

        # img (token-major, for LN stats) + imgT halves on sync queue
        x_ts = []
        for t in range(NT):
            x_t = ln_p.tile([128, DIM], F32, name="x_t", tag="x_t", bufs=3)
            nc.sync.dma_start(out=x_t, in_=img[t * 128:(t + 1) * 128, :])
            x_ts.append(x_t)
        xiT = [xi_p.tile([128, NTOK], F32, name=f"xiT{c}", tag=f"xiT{c}")
               for c in range(NC_)]
        for qb in range(NQB):
            for c in range(NC_):
                nc.sync.dma_start(
                    out=xiT[c][:, qb * QB:(qb + 1) * QB],
                    in_=imgT[c * 128:(c + 1) * 128, qb * QB:(qb + 1) * QB])
        tb = ln_p.tile([1, DIM], F32, name="tb", tag="tb", bufs=1)
        nc.sync.dma_start(out=tb, in_=tab)

        wq = []
        for c in range(NC_):
            w = wq_p.tile([128, 3 * INNER], F32R, name=f"wq{c}", tag=f"wq{c}")
            nc.gpsimd.dma_start(out=w, in_=w_qkv[c * 128:(c + 1) * 128, :].bitcast(F32R))
            wq.append(w)
        wt = []
        for c in range(NC_):
            w = wq_p.tile([128, 2 * INNER], F32R, name=f"wt{c}", tag=f"wt{c}")
            nc.gpsimd.dma_start(out=w, in_=w_tab[c * 128:(c + 1) * 128, :].bitcast(F32R))
            wt.append(w)
        wo_f = []
        for c in range(NC_):
            w = wq_p.tile([128, DIM], F32, name=f"wof{c}", tag=f"wof{c}")
            nc.gpsimd.dma_start(out=w, in_=w_out[c * 128:(c + 1) * 128, :])
            wo_f.append(w)
        nc.gpsimd.dma_start(out=bout_bc, in_=b_out.to_broadcast([128, DIM]))

        # ---------------- persistent activations ----------------
        qT = [qkT_p.tile([128, NTOK], F32R, name=f"qT{c}", tag=f"qT{c}")
              for c in range(NC_)]
        kT = [qkT_p.tile([128, NTOK], F32R, name=f"kT{c}", tag=f"kT{c}")
              for c in range(NC_)]
        v_sb = [v_p.tile([128, 528], BF16, name=f"v{t}", tag=f"v{t}")
                for t in range(NT)]
        outuT = [outuT_p.tile([128, NTOK], BF16, name=f"ouT{c}", tag=f"ouT{c}")
                 for c in range(NC_)]
        wo = [wo_p.tile([128, DIM], BF16, name=f"wo{c}", tag=f"wo{c}")
              for c in range(NC_)]

        k_tT = small_p.tile([128, NC_], F32R, name="k_tT")
        krow = small_p.tile([1, INNER], F32, name="krow")
        bd = [[small_p.tile([128, 97], F32R, name=f"bd{g}{i}")
               for i in range(2)] for g in range(2)]
        v_t_aug = small_p.tile([128, 528], BF16, name="v_t_aug")
        # denominator collectors; head 4g+j lives at partition 32j
        dcol = [small_p.tile([128, NTOK], F32, name=f"dcol{g}") for g in range(2)]
        rtmp = small_p.tile([128, QB], F32, name="rtmp")
        for g in range(2):
            nc.gpsimd.memset(dcol[g], 1.0)
        tabexp = small_p.tile([128, 2 * NTOK], BF16, name="tabexp")
        # LN stat rows: msrow = mean*rstd, rsrow = rstd (token-major rows)
        msrow = small_p.tile([1, NTOK], F32R, name="msrow")
        rsrow = small_p.tile([1, NTOK], F32R, name="rsrow")
        rbf = small_p.tile([128, QB], BF16, name="rbf")

        xnT = [xi_p.tile([128, NTOK], F32R, name=f"xnT{c}", tag=f"xnT{c}")
               for c in range(NC_)]

        # ---------------- emission helpers ----------------
        def emit_ln_stats(t):
            x_t = x_ts[t]
            stats = ln_p.tile([128, 6], F32, name="stats", tag="stats")
            nc.vector.bn_stats(out=stats, in_=x_t)
            mv = ln_p.tile([128, 2], F32, name="mv", tag="mv")
            nc.vector.bn_aggr(out=mv, in_=stats)
            sd = ln_p.tile([128, 1], F32, name="sd", tag="sd")
            nc.scalar.activation(out=sd, in_=mv[:, 1:2], func=AF.Sqrt,
                                 bias=eps_t, scale=1.0)
            rstd = ln_p.tile([128, 1], F32, name="rstd", tag="rstd")
            nc.vector.reciprocal(out=rstd, in_=sd)
            msc = ln_p.tile([128, 1], F32, name="msc", tag="msc")
            nc.vector.tensor_tensor(out=msc, in0=mv[:, 0:1], in1=rstd,
                                    op=OP.mult)
            pt = o_tile([1, 256], "plnr")
            nc.tensor.transpose(out=pt[0:1, 0:128], in_=rstd, identity=ident)
            nc.tensor.transpose(out=pt[0:1, 128:256], in_=msc, identity=ident)
            ts_ = slice(t * 128, (t + 1) * 128)
            nc.vector.tensor_copy(out=rsrow[0:1, ts_], in_=pt[0:1, 0:128])
            nc.vector.tensor_copy(out=msrow[0:1, ts_], in_=pt[0:1, 128:256])

        def emit_xnT_half(qb):
            # xnT[:, qb] = imgT * rstd_bc - musc_bc  (ln_w/ln_b are folded
            # into the weights on the host)
            qs = slice(qb * QB, (qb + 1) * QB)
            rb = o_tile([128, QB], "rbps")
            nc.tensor.matmul(rb, lhsT=ones_fr[0:1, :],
                             rhs=rsrow[0:1, qs],
                             start=True, stop=True, tile_position=(0, 0))
            mb = o_tile([128, QB], "mbps")
            nc.tensor.matmul(mb, lhsT=ones_fr[0:1, :],
                             rhs=msrow[0:1, qs],
                             start=True, stop=True, tile_position=(0, 0))
            for c in range(NC_):
                nc.vector.tensor_tensor(out=xnT[c][:, qs], in0=xiT[c][:, qs],
                                        in1=rb, op=OP.mult)
                nc.vector.tensor_tensor(out=xnT[c][:, qs], in0=xnT[c][:, qs],
                                        in1=mb, op=OP.subtract)

        qkctr = [0]

        def emit_qk_pair(m1, m2, qb):
            # one q block for two qkv column blocks (one psum bank each)
            ps = big_tile("psqk")
            eng = nc.scalar if qkctr[0] < 2 else nc.vector
            qkctr[0] += 1
            for idx, m in enumerate((m1, m2)):
                half = ps[:, idx * QB:(idx + 1) * QB]
                for kc in range(NC_):
                    nc.tensor.matmul(
                        half,
                        lhsT=wq[kc][:, m * 128:(m + 1) * 128],
                        rhs=xnT[kc][:, qb * QB:(qb + 1) * QB],
                        start=(kc == 0), stop=(kc == NC_ - 1))
                dst = qT[m] if m < 4 else kT[m - 4]
                if eng is nc.scalar:
                    nc.scalar.activation(
                        out=dst[:, qb * QB:(qb + 1) * QB], in_=half,
                        func=AF.Identity, bias=cqkc[:, m:m + 1], scale=1.0)
                else:
                    nc.vector.tensor_scalar(
                        out=dst[:, qb * QB:(qb + 1) * QB], in0=half,
                        scalar1=cqkc[:, m:m + 1], scalar2=None, op0=OP.add)

        def emit_v(t):
            pv = o_tile([128, QB], "psv")
            for kc in range(NC_):
                nc.tensor.matmul(
                    pv,
                    lhsT=xnT[kc][:, t * 128:(t + 1) * 128],
                    rhs=wq[kc][:, 2 * INNER:3 * INNER],
                    start=(kc == 0), stop=(kc == NC_ - 1))
            vdst = v_sb[t].rearrange("p (h s) -> p h s", s=66)
            nc.vector.tensor_tensor(
                out=vdst[:, :, 0:64],
                in0=pv.rearrange("p (h d) -> p h d", d=64),
                in1=cv_bc.rearrange("p (h d) -> p h d", d=64), op=OP.add)
            nc.vector.tensor_copy(
                out=vdst[:, :, 64:65],
                in_=ones8.rearrange("p (h o) -> p h o", o=1))

        ae = {}
        aectr = [0]

        def emit_dots(qb, hp):
            for kp in range(4):
                psA = big_tile("psd0")
                psB = big_tile("psd1")
                for i, kt in enumerate((2 * kp, 2 * kp + 1)):
                    for hh, ps in ((0, psA), (1, psB)):
                        hb = hh * 64
                        nc.tensor.matmul(
                            ps[:, i * QB:(i + 1) * QB],
                            lhsT=kT[hp][hb:hb + 64, kt * 128:(kt + 1) * 128],
                            rhs=qT[hp][hb:hb + 64, qb * QB:(qb + 1) * QB],
                            start=True, stop=True)
                for hh, ps in ((0, psA), (1, psB)):
                    t_ae = ae_p.tile([128, 2 * QB], BF16, name="ae",
                                     tag=f"ae{aectr[0] % 10}")
                    aectr[0] += 1
                    nc.scalar.activation(out=t_ae, in_=ps, func=AF.Exp,
                                         scale=SCALE)
                    ae[(qb, hp, hh, kp)] = t_ae

        def emit_attnv(qb, hp):
            for hh in range(2):
                h = 2 * hp + hh
                g, j = h // 4, h % 4
                po = o_tile([65, QB], "po")
                for kt in range(NT):
                    t_ae = ae[(qb, hp, hh, kt // 2)]
                    nc.tensor.matmul(
                        po,
                        lhsT=v_sb[kt][:, 66 * h:66 * h + 65],
                        rhs=t_ae[:, (kt % 2) * QB:(kt % 2 + 1) * QB],
                        start=(kt == 0), stop=False)
                nc.tensor.matmul(
                    po,
                    lhsT=v_t_aug[32 * j:32 * j + 1, 66 * h:66 * h + 65],
                    rhs=tabexp[32 * j:32 * j + 1,
                               (2 * qb + g) * QB:(2 * qb + g + 1) * QB],
                    start=False, stop=True,
                    tile_position=(32 * j, 0))
                nc.vector.tensor_copy(
                    out=outuT[hp][64 * hh:64 * hh + 64, qb * QB:(qb + 1) * QB],
                    in_=po[0:64, :])
                nc.vector.tensor_copy(
                    out=dcol[g][32 * j:32 * j + 1, qb * QB:(qb + 1) * QB],
                    in_=po[64:65, :])

        def emit_norm_pair(qb, hp):
            # normalize head pair hp of q block qb: 1/d broadcast down
            # partitions via K=1 f32r ones-matmuls, multiply on DVE
            qs = slice(qb * QB, (qb + 1) * QB)
            g = hp // 2
            r0 = 64 * (hp % 2)
            r1 = r0 + 32
            # custom-DVE recip misbehaves at nonzero base partition: always
            # run rows 0:97 (cost scales with columns, not partitions);
            # rows of the other pair are stale-but-unused at hp even.
            nc.vector.reciprocal_approx_fast(out=rtmp[0:97, :],
                                             in_=dcol[g][0:97, qs])
            nc.vector.tensor_copy(out=rbf[0:97, :], in_=rtmp[0:97, :])
            bc0 = o_tile([64, QB], "bc0")
            nc.tensor.matmul(bc0, lhsT=ones_row[r0:r0 + 1, :],
                             rhs=rbf[r0:r0 + 1, :],
                             start=True, stop=True, tile_position=(r0, 0))
            bc1 = o_tile([64, QB], "bc1")
            nc.tensor.matmul(bc1, lhsT=ones_row[r1:r1 + 1, :],
                             rhs=rbf[r1:r1 + 1, :],
                             start=True, stop=True, tile_position=(r1, 0))
            nc.vector.tensor_tensor(out=outuT[hp][0:64, qs],
                                    in0=outuT[hp][0:64, qs], in1=bc0,
                                    op=OP.mult)
            nc.vector.tensor_tensor(out=outuT[hp][64:128, qs],
                                    in0=outuT[hp][64:128, qs], in1=bc1,
                                    op=OP.mult)

        def emit_tabdots(qb, g):
            ps = o_tile([97, QB], "pstd")
            for i in range(2):
                nc.tensor.matmul(
                    ps,
                    lhsT=bd[g][i],
                    rhs=qT[2 * g + i][:, qb * QB:(qb + 1) * QB],
                    start=(i == 0), stop=(i == 1))
            nc.scalar.activation(
                out=tabexp[0:97, (2 * qb + g) * QB:(2 * qb + g + 1) * QB],
                in_=ps, func=AF.Exp, scale=SCALE)

        def emit_final(qb):
            for t in range(4 * qb, 4 * qb + 4):
                pf = o_tile([128, DIM], "pf")
                for c in range(NC_):
                    nc.tensor.matmul(
                        pf,
                        lhsT=outuT[c][:, t * 128:(t + 1) * 128],
                        rhs=wo[c],
                        start=(c == 0), stop=(c == NC_ - 1))
                fo = fo_p.tile([128, DIM], F32, name="fo", tag="fo")
                nc.vector.tensor_tensor(out=fo, in0=pf, in1=bout_bc, op=OP.add)
                nc.sync.dma_start(out=out_d[t * 128:(t + 1) * 128, :], in_=fo)

        def emit_tab_setup():
            # LN(tab) stats only (weights folded); tnT; k_t/v_t rows (+bias);
            # k_tT; block-diag bd; v_t_aug
            tstats = ln_p.tile([1, 6], F32, name="tstats", tag="tstats")
            nc.vector.bn_stats(out=tstats, in_=tb)
            tmv = ln_p.tile([1, 2], F32, name="tmv", tag="tmv")
            nc.vector.bn_aggr(out=tmv, in_=tstats)
            tsd = ln_p.tile([1, 1], F32, name="tsd", tag="tsd")
            nc.scalar.activation(out=tsd, in_=tmv[:, 1:2], func=AF.Sqrt,
                                 bias=eps_t[0:1], scale=1.0)
            trstd = ln_p.tile([1, 1], F32, name="trstd", tag="trstd")
            nc.vector.reciprocal(out=trstd, in_=tsd)
            tn = ln_p.tile([1, DIM], F32, name="tn", tag="tn", bufs=1)
            nc.vector.tensor_scalar(out=tn, in0=tb, scalar1=tmv[:, 0:1],
                                    scalar2=trstd, op0=OP.subtract, op1=OP.mult)

            tnT = ln_p.tile([128, NC_], F32R, name="tnT", tag="tnT", bufs=1)
            for c in range(NC_):
                pt = o_tile([128, 1], "ptn")
                nc.tensor.transpose(out=pt, in_=tn[0:1, c * 128:(c + 1) * 128],
                                    identity=ident[0:1, 0:1])
                nc.vector.tensor_copy(out=tnT[:, c:c + 1], in_=pt)

            # k_t row [1, 512] (+ ln_b bias) then transpose into k_tT columns
            ps_kt = o_tile([1, INNER], "pskt")
            for kc in range(NC_):
                nc.tensor.matmul(
                    ps_kt,
                    lhsT=tnT[:, kc:kc + 1],
                    rhs=wt[kc][:, 0:INNER],
                    start=(kc == 0), stop=(kc == NC_ - 1))
            nc.vector.tensor_tensor(out=krow, in0=ps_kt,
                                    in1=ctab_sb[0:1, 0:INNER], op=OP.add)
            for c in range(NC_):
                pk = o_tile([128, 1], "pk")
                nc.tensor.transpose(out=pk, in_=krow[0:1, c * 128:(c + 1) * 128],
                                    identity=ident[0:1, 0:1])
                nc.vector.tensor_copy(out=k_tT[:, c:c + 1], in_=pk)

            for g in range(2):
                for i in range(2):
                    nc.vector.memset(bd[g][i].bitcast(F32), 0.0)
                    c = 2 * g + i
                    nc.vector.tensor_copy(
                        out=bd[g][i][0:64, 32 * (2 * i):32 * (2 * i) + 1],
                        in_=k_tT[0:64, c:c + 1])
                    nc.vector.tensor_copy(
                        out=bd[g][i][64:128, 32 * (2 * i + 1):32 * (2 * i + 1) + 1],
                        in_=k_tT[64:128, c:c + 1])

            # v_t row (+ bias) then augmented + replicated
            ps_vt = o_tile([1, INNER], "psvt")
            for kc in range(NC_):
                nc.tensor.matmul(
                    ps_vt,
                    lhsT=tnT[:, kc:kc + 1],
                    rhs=wt[kc][:, INNER:2 * INNER],
                    start=(kc == 0), stop=(kc == NC_ - 1))
            vta = v_t_aug[0:1, :].rearrange("p (h s) -> p h s", s=66)
            nc.vector.tensor_tensor(
                out=vta[:, :, 0:64],
                in0=ps_vt.rearrange("p (h d) -> p h d", d=64),
                in1=ctab_sb[0:1, INNER:2 * INNER].rearrange(
                    "p (h d) -> p h d", d=64), op=OP.add)
            nc.vector.tensor_copy(
                out=vta[:, :, 64:65],
                in_=ones8[0:1, :].rearrange("p (h o) -> p h o", o=1))
            nc.gpsimd.partition_broadcast(out_ap=v_t_aug, in_ap=v_t_aug[0:1, :])

        # ---------------- debug dumps ----------------
        def dump(dst, row, src_ap, cast=False):
            if dbg is None:
                return
            if cast:
                st = dbg_p.tile([128, QB], F32, name="dbgst", tag="dbgst")
                nc.vector.tensor_copy(out=st, in_=src_ap)
                src_ap = st
            nc.sync.dma_start(out=dst[row:row + 128, :], in_=src_ap)

        # ---------------- emission schedule ----------------
        # dots(qb, hp) needs kT[hp] for ALL 1024 key tokens but qT[hp] only
        # for its own q block: pair the k-halves across q blocks.
        for t in range(4):
            emit_ln_stats(t)
        emit_xnT_half(0)
        emit_qk_pair(0, 4, 0)
        for t in range(4, NT):
            emit_ln_stats(t)
        emit_xnT_half(1)
        emit_qk_pair(4, 1, 1)           # kT0 qb1 + qT1 qb1
        if dbg is not None:
            dump(dbg["dbg_qk"], 0, qT[0][:, 0:QB].bitcast(F32))
            dump(dbg["dbg_qk"], 256, kT[0][:, 0:QB].bitcast(F32))
            dump(dbg["dbg_qk"], 512, xnT[0][:, 0:QB].bitcast(F32))
        emit_dots(0, 0)                 # exp stream starts here
        if dbg is not None:
            for kp in range(4):
                dump(dbg["dbg_ae"], 128 * kp, ae[(0, 0, 0, kp)][:, 0:QB],
                     cast=True)
                dump(dbg["dbg_ae"], 512 + 128 * kp, ae[(0, 0, 1, kp)][:, 0:QB],
                     cast=True)
        emit_qk_pair(1, 5, 0)
        emit_qk_pair(5, 2, 1)           # kT1 qb1 + qT2 qb1
        for t in range(NT):
            emit_v(t)
        emit_tab_setup()
        emit_tabdots(0, 0)
        if dbg is not None:
            dump(dbg["dbg_qk"], 768, v_sb[0][:, 0:QB], cast=True)
            dump(dbg["dbg_qk"], 896, tabexp[:, 0:QB], cast=True)
        emit_attnv(0, 0)
        emit_norm_pair(0, 0)
        emit_dots(0, 1)
        emit_qk_pair(2, 6, 0)
        emit_qk_pair(6, 3, 1)           # kT2 qb1 + qT3 qb1
        emit_attnv(0, 1)
        if dbg is not None:
            dump(dbg["dbg_att"], 0, outuT[1][:, 0:QB], cast=True)
            dump(dbg["dbg_att"], 256, dcol[0][:, 0:QB])
        emit_norm_pair(0, 1)
        if dbg is not None:
            dump(dbg["dbg_att"], 384, outuT[0][:, 0:QB], cast=True)
            dump(dbg["dbg_att"], 512, outuT[1][:, 0:QB], cast=True)
        emit_dots(0, 2)
        emit_qk_pair(3, 7, 0)
        emit_qk_pair(7, 0, 1)           # kT3 qb1 + qT0 qb1
        emit_tabdots(0, 1)
        # w_out -> bf16 casts (DVE, off the critical path)
        for c in range(NC_):
            nc.vector.tensor_copy(out=wo[c], in_=wo_f[c])
        scoped.close()                  # free wq/wt/wo_f, imgT/xnT, LN pools
        emit_attnv(0, 2)
        emit_norm_pair(0, 2)
        emit_dots(0, 3)
        emit_attnv(0, 3)
        emit_norm_pair(0, 3)
        emit_dots(1, 0)
        emit_tabdots(1, 0)
        emit_final(0)
        emit_attnv(1, 0)
        emit_norm_pair(1, 0)
        emit_dots(1, 1)
        emit_attnv(1, 1)
        emit_norm_pair(1, 1)
        emit_dots(1, 2)
        emit_tabdots(1, 1)
        emit_attnv(1, 2)
        emit_norm_pair(1, 2)
        emit_dots(1, 3)
        emit_attnv(1, 3)
        emit_norm_pair(1, 3)
        emit_final(1)


_CACHED_NC = None


def _host_inputs(inputs):
    img = np.ascontiguousarray(np.asarray(inputs["img"], dtype=np.float32))
    tab = np.ascontiguousarray(np.asarray(inputs["tab"], dtype=np.float32))
    w_qkv = np.asarray(inputs["w_qkv"], dtype=np.float32)
    w_tab_qkv = np.asarray(inputs["w_tab_qkv"], dtype=np.float32)
    w_out = np.ascontiguousarray(np.asarray(inputs["w_out"], dtype=np.float32))
    b_out = np.asarray(inputs["b_out"], dtype=np.float32).reshape(1, DIM)
    ln_w = np.asarray(inputs["ln_w"], dtype=np.float32).reshape(DIM)
    ln_b = np.asarray(inputs["ln_b"], dtype=np.float32).reshape(DIM)

    w_qkv_f = np.ascontiguousarray(ln_w[:, None] * w_qkv)
    w_tab_kv = w_tab_qkv[:, INNER:3 * INNER]
    w_tab_f = np.ascontiguousarray(ln_w[:, None] * w_tab_kv)
    cqkv = ln_b @ w_qkv                       # [1536]
    cqkvT = np.ascontiguousarray(cqkv.reshape(12, 128).T)
    cv = np.ascontiguousarray(cqkv[2 * INNER:3 * INNER].reshape(1, INNER))
    ctab = np.ascontiguousarray((ln_b @ w_tab_kv).reshape(1, 2 * INNER))
    return img, tab, w_qkv_f, w_tab_f, w_out, b_out, cqkvT, cv, ctab


def kernel(**inputs):
    global _CACHED_NC
    img, tab, w_qkv_f, w_tab_f, w_out, b_out, cqkvT, cv, ctab = \
        _host_inputs(inputs)

    if _CACHED_NC is None:
        _CACHED_NC = build_program()
    nc = _CACHED_NC

    in_maps = []
    for b in range(N_CORES):
        in_maps.append({
            "img_s": np.ascontiguousarray(img[b]),
            "imgT_s": np.ascontiguousarray(img[b].T),
            "tab_s": np.ascontiguousarray(tab[b]),
            "w_qkv": w_qkv_f,
            "w_tab": w_tab_f,
            "w_out": w_out,
            "b_out": b_out,
            "cqkvT_s": cqkvT,
            "cv_s": cv,
            "ctab_s": ctab,
        })

    res = bass_utils.run_bass_kernel_spmd(nc, in_maps, core_ids=list(range(N_CORES)))
    out = np.stack([res.results[c]["out_s"] for c in range(N_CORES)], axis=0)
    return out.astype(np.float32)


if __name__ == "__main__":
    d = np.load("/root/problem/ref_data.npz")
    ins = {k: d[k] for k in ("img", "tab", "w_qkv", "w_tab_qkv", "w_out",
                             "b_out", "ln_w", "ln_b")}
    actual = kernel(**ins)
    expected = d["expected"]
    err = np.abs(actual - expected).max()
    rel = err / np.abs(expected).max()
    print("absmax err:", err, "rel:", rel)
